# revision 13
# baseline (speedup 1.0000x reference)
"""Trainium2 Bass kernel for nn_AugmentableSVSAlgorithm (scatter_memory).

Reference semantics (see problem):
  per-frame recurrence over T=2000 frames with carry (ht, lt) [128,160]:
      th = sigmoid((x - ht - d_hot) * hc);  tl = sigmoid((lt - x - d_hot) * hc)
      ht' = ht + d_open if x > ht else ht - d_close
      lt' = lt - d_open if lt > x else lt + d_close
      hot = th + tl
  then out = relu(1 - conv3x3(1 - hot, k, pad=1)).

Implementation strategy (8 NeuronCores, SPMD):
  * H-split sharding: core c owns rows [16c, 16c+16), processes rows
    [16c-1, 16c+17) (1 halo row each side). Out-of-range halo rows and the
    W-padding columns are filled with x = +1e6 on the host, which saturates
    both sigmoids so hot == 1.0 exactly — this reproduces the reference
    conv's zero-padding of (1 - hot) uniformly with no edge cases.
  * State is paired per pixel as S = [h | L] with L = -lt; then both
    thresholds obey the SAME update S' = select(x2 > S, S+d_open, S-d_close)
    with x2 = [x | -x].  A fused custom VectorE op executes one full time
    step in ONE instruction; branch decisions and state arithmetic are
    bit-exact vs the reference (mandatory: the thresholds gate near-step
    sigmoids, so a 1-ulp state divergence can flip outputs by ~1).
  * Per core the 18x160 pixels (plus duplicated page-halo columns) are laid
    out as 126 partitions = (h:18, wb:7) x 25 free (pages of 23 real cols +
    2 halo cols), so the 3x3 conv never crosses a page boundary in the free
    dim; the H direction lives on partitions and is done by TensorE with
    banded 126x126 matrices (one per kernel column), accumulated in PSUM.
  * Sigmoids batched on ScalarE; elementwise prep/combine on GpSimd;
    everything overlaps behind the 2000-instruction VectorE chain.
"""

import numpy as np

T, H, W = 2000, 128, 160
NCORES = 8
ROWS = 18            # rows processed per core (16 own + 2 halo)
NB = 7               # W pages
PW = 25              # stored page width (23 real + 2 halo)
STRIDE = 23          # page stride in real-w
P = ROWS * NB        # 126 partitions
WIN = 100            # time steps per window
CH = 20              # conv chunk (steps per PSUM accumulation group)
PAD = np.float32(1.0e6)

_OP = None


def _register_op():
    global _OP
    if _OP is not None:
        return _OP
    from concourse import dve_ops
    from concourse.dve_spec import Spec, Src0, Src1, C0, C1, select, lower
    from concourse.dve_uop import DveOpSpec

    name = "SVS_UPDATE_ANT"
    for o in dve_ops.OPS:
        if o.name == name:
            _OP = o
            return o
    spec = Spec(
        body=select(Src0 > Src1, Src1 + C0, Src1 - C1),
        reference=lambda in0, in1, c0, c1, c2: np.where(
            in0 > in1,
            (in1 + np.float32(c0)).astype(np.float32),
            (in1 - np.float32(c1)).astype(np.float32),
        ).astype(np.float32),
    )
    opcode = dve_ops._CUSTOM_DVE_ROW_BASE + len(dve_ops.OPS)
    shas = {}
    for ver in ("v3", "v4"):
        uops = lower(spec, ver=ver)
        shas[ver] = DveOpSpec(name=name, opcode=opcode, uops=uops, rd1_en=True).sha(ver)
    op = dve_ops.DveOp(name, spec, subdim=False, uops_sha=shas)
    dve_ops.OPS.append(op)
    dve_ops._SUB_OPCODE_FOR_NAME[name] = opcode
    dve_ops.CUSTOM_DVE_SPECS[name] = spec
    _OP = op
    return op


def _build_program(d_open, d_close, hc, hbias, relu_bias):
    """One SPMD Bass program (same instruction stream on all 8 cores)."""
    from concourse import mybir, tile, bacc

    op = _register_op()
    nc = bacc.Bacc("TRN2", target_bir_lowering=False, debug=False,
                   num_devices=NCORES)
    f32 = mybir.dt.float32
    xp_d = nc.dram_tensor("xp", [P, T, PW], f32, kind="ExternalInput").ap()
    s0_d = nc.dram_tensor("s0", [P, 2 * PW], f32, kind="ExternalInput").ap()
    band_d = nc.dram_tensor("band", [3, P, P], f32, kind="ExternalInput").ap()
    out_d = nc.dram_tensor("out", [P, T, STRIDE], f32, kind="ExternalOutput").ap()

    Sig = mybir.ActivationFunctionType.Sigmoid
    Relu = mybir.ActivationFunctionType.Relu
    FD = 2 * PW
    NW = T // WIN

    with tile.TileContext(nc) as tc:
        with (
            tc.tile_pool(name="consts", bufs=1) as cpool,
            tc.tile_pool(name="xraw", bufs=2) as xrpool,
            tc.tile_pool(name="x2", bufs=2) as x2pool,
            tc.tile_pool(name="traj", bufs=2) as tpool,
            tc.tile_pool(name="eth", bufs=2) as epool,
            tc.tile_pool(name="hot", bufs=2) as hpool,
            tc.tile_pool(name="outw", bufs=2) as opool,
            tc.tile_pool(name="psum", bufs=4, space="PSUM") as ppool,
        ):
            bands = cpool.tile([P, 3 * P], f32)
            nc.sync.dma_start(bands[:].rearrange("p (d q) -> p d q", d=3),
                              band_d.rearrange("d p q -> p d q"))
            hbias_t = cpool.tile([P, 1], f32)
            nc.gpsimd.memset(hbias_t[:], hbias)
            rbias_t = cpool.tile([P, 1], f32)
            nc.gpsimd.memset(rbias_t[:], relu_bias)

            trajs = []
            for w in range(NW):
                traj = tpool.tile([P, FD * (WIN + 1)], f32, tag="traj")
                trajs.append(traj)
                xr = xrpool.tile([P, PW * WIN], f32, tag="xraw")
                nc.sync.dma_start(
                    xr[:].rearrange("p (t f) -> p t f", t=WIN),
                    xp_d[:, w * WIN:(w + 1) * WIN, :],
                )
                x2 = x2pool.tile([P, FD * WIN], f32, tag="x2")
                x2v = x2[:].rearrange("p (t r f) -> p t r f", r=2, f=PW)
                xrv = xr[:].rearrange("p (t f) -> p t f", t=WIN)
                nc.gpsimd.tensor_copy(x2v[:, :, 0, :], xrv)
                nc.gpsimd.tensor_scalar_mul(x2v[:, :, 1, :], xrv, -1.0)
                # carry into slot 0
                if w == 0:
                    nc.sync.dma_start(traj[:, 0:FD], s0_d[:])
                else:
                    nc.vector.tensor_copy(
                        traj[:, 0:FD], trajs[w - 1][:, FD * WIN:FD * (WIN + 1)]
                    )
                # the chain: one fused op per time step
                for i in range(WIN):
                    nc.vector._custom_dve(
                        op,
                        out=traj[:, FD * (i + 1):FD * (i + 2)],
                        in0=x2[:, FD * i:FD * (i + 1)],
                        in1=traj[:, FD * i:FD * (i + 1)],
                        s0=d_open,
                        s1=d_close,
                    )
                # E = x2 - S_pre  (pre-update states = slots 0..WIN-1)
                eth = epool.tile([P, FD * WIN], f32, tag="eth")
                nc.gpsimd.tensor_sub(eth[:], x2[:], traj[:, 0:FD * WIN])
                # TH = sigmoid(hc*E + hbias), in place
                nc.scalar.activation(eth[:], eth[:], Sig, bias=hbias_t[:], scale=hc)
                # hot = th + tl
                hot = hpool.tile([P, PW * WIN], f32, tag="hot")
                ethv = eth[:].rearrange("p (t r f) -> p t r f", r=2, f=PW)
                nc.gpsimd.tensor_add(
                    hot[:].rearrange("p (t f) -> p t f", t=WIN),
                    ethv[:, :, 0, :],
                    ethv[:, :, 1, :],
                )
                # conv: 3 banded matmuls per chunk into PSUM, then relu on ACT
                outw = opool.tile([P, STRIDE * WIN], f32, tag="outw")
                for cstart in range(0, WIN, CH):
                    ps = ppool.tile([P, STRIDE * CH], f32, tag="ps")
                    hchunk = hot[:, PW * cstart:PW * (cstart + CH)].rearrange(
                        "p (t f) -> p t f", t=CH
                    )
                    for j, dx in enumerate((-1, 0, 1)):
                        rhs = hchunk[:, :, 1 + dx:1 + dx + STRIDE]
                        nc.tensor.matmul(
                            ps[:].rearrange("p (t f) -> p t f", t=CH),
                            bands[:, (dx + 1) * P:(dx + 2) * P],
                            rhs,
                            start=(j == 0),
                            stop=(j == 2),
                        )
                    nc.scalar.activation(
                        outw[:, STRIDE * cstart:STRIDE * (cstart + CH)],
                        ps[:], Relu, bias=rbias_t[:], scale=1.0,
                    )
                # out DMA: full staging tile (host drops halo rows / pad cols)
                ws = w * WIN
                nc.sync.dma_start(
                    out_d[:, ws:ws + WIN, :],
                    outw[:].rearrange("p (t j) -> p t j", j=STRIDE),
                )
    nc.compile()
    return nc


_PROG_CACHE = {}


def _get_program(key, d_open, d_close, hc, hbias, relu_bias):
    if key not in _PROG_CACHE:
        _PROG_CACHE[key] = _build_program(d_open, d_close, hc, hbias, relu_bias)
    return _PROG_CACHE[key]


def _prep_inputs(x, params, ht0, lt0, kern):
    """Build per-core input maps (host-side sharding)."""
    x = np.ascontiguousarray(x.reshape(T, H, W).astype(np.float32))
    ht0 = ht0.astype(np.float32)
    lt0 = lt0.astype(np.float32)
    kern = kern.astype(np.float32)

    # padded frame: rows [-1, H], cols [-1, W+1], pad value 1e6
    xp = np.full((T, H + 2, W + 3), PAD, np.float32)
    xp[:, 1:H + 1, 1:W + 1] = x
    hp = np.zeros((H + 2, W + 3), np.float32)
    hp[1:H + 1, 1:W + 1] = ht0
    lp = np.zeros((H + 2, W + 3), np.float32)
    lp[1:H + 1, 1:W + 1] = -lt0

    # band matrices: band[dx][p_in, p_out] = k[h_in-h_out+1, dx+1]
    band = np.zeros((3, P, P), np.float32)
    for dxi in range(3):
        for h_out in range(ROWS):
            for dy in (-1, 0, 1):
                h_in = h_out + dy
                if 0 <= h_in < ROWS:
                    for wb in range(NB):
                        band[dxi, h_in * NB + wb, h_out * NB + wb] = kern[dy + 1, dxi]

    in_maps = []
    for c in range(NCORES):
        r0 = 16 * c  # padded-row index of first processed row (= global 16c-1)
        # pages: (h, wb) -> padded cols [23*wb, 23*wb+25)
        xc = np.empty((ROWS, NB, T, PW), np.float32)
        sc = np.empty((ROWS, NB, 2 * PW), np.float32)
        for wb in range(NB):
            c0 = STRIDE * wb
            xc[:, wb] = xp[:, r0:r0 + ROWS, c0:c0 + PW].transpose(1, 0, 2)
            sc[:, wb, 0:PW] = hp[r0:r0 + ROWS, c0:c0 + PW]
            sc[:, wb, PW:2 * PW] = lp[r0:r0 + ROWS, c0:c0 + PW]
        in_maps.append({
            "xp": np.ascontiguousarray(xc.reshape(P, T, PW)),
            "s0": np.ascontiguousarray(sc.reshape(P, 2 * PW)),
            "band": band,
        })
    return in_maps


TRACE = False        # test-harness hook: profile the SPMD run
LAST_RESULT = None


def kernel(x, params, ht0, lt0, kernel):
    global LAST_RESULT
    from concourse.bass_utils import run_bass_kernel_spmd

    p = np.asarray(params, np.float32)
    d_close, d_open, d_hot, hc = (float(p[0]), float(p[1]), float(p[2]), float(p[3]))
    kern = np.asarray(kernel, np.float32)
    hbias = float(np.float32(-np.float32(d_hot) * np.float32(hc)))
    relu_bias = float(np.float32(1.0) - np.float32(kern.sum()))

    key = (d_close, d_open, d_hot, hc, kern.tobytes())
    nc = _get_program(key, d_open, d_close, hc, hbias, relu_bias)
    in_maps = _prep_inputs(np.asarray(x), p, np.asarray(ht0), np.asarray(lt0), kern)
    r = run_bass_kernel_spmd(nc, in_maps, list(range(NCORES)), trace=TRACE)
    LAST_RESULT = r
    res = r.results
    out = np.empty((T, H, W), np.float32)
    for c in range(NCORES):
        out[:, 16 * c:16 * (c + 1), :] = _assemble(res[c]["out"])
    return out.reshape(T, 1, H, W).astype(np.float32)


def _assemble(raw):
    """[P, T, STRIDE] staging -> [T, 16, W] (drop halo rows h=0,17, pad cols)."""
    v = raw.reshape(ROWS, NB, T, STRIDE)[1:17]  # own rows
    full = v.transpose(2, 0, 1, 3).reshape(T, 16, NB * STRIDE)
    return full[:, :, :W]


# revision 14
# speedup vs baseline: 3.1550x; 3.1550x over previous
"""Trainium2 Bass kernel for nn_AugmentableSVSAlgorithm (scatter_memory).

Reference semantics:
  per-frame recurrence over T=2000 frames with carry (ht, lt) [128,160]:
      th = sigmoid((x - ht - d_hot) * hc);  tl = sigmoid((lt - x - d_hot) * hc)
      ht' = ht + d_open if x > ht else ht - d_close
      lt' = lt - d_open if lt > x else lt + d_close
      hot = th + tl
  then out = relu(1 - conv3x3(1 - hot, k, pad=1)).

Implementation (8 NeuronCores, SPMD, H-split sharding):
  * Core c owns rows [16c, 16c+16), processes rows [16c-1, 16c+17) (halo).
    Out-of-range halo rows and W-pad columns carry x = +1e6, which saturates
    both sigmoids so hot == 1.0 exactly — reproducing the reference conv's
    zero-padding of (1 - hot) with no edge cases anywhere.
  * State pairs S = [h | L], L = -lt: both thresholds share the update
    S' = select(x2 > S, S + d_open, S - d_close), x2 = [x | -x] (interleaved
    on the host). One fused custom VectorE op per time step — branch
    decisions and state arithmetic bit-exact vs the reference.
  * Layout per core: 126 partitions = (h:18, wb:7), pages of 25 cols
    (23-col stride + 2 halo cols) so the conv never crosses pages in the
    free dim. H direction on partitions via TensorE banded matrices.
  * VectorE runs ONLY the chain. E = hc*x2 - hc*S is produced by TensorE
    (scaled identity matmuls) into PSUM; ScalarE applies sigmoid straight
    from PSUM (fp16 out). The th+tl add is folded into the conv by
    linearity (6 banded fp16 matmuls per chunk). GpSimd does nothing —
    it would contend with VectorE for the shared SBUF port.
"""

import numpy as np

T, H, W = 2000, 128, 160
NCORES = 8
ROWS = 18            # rows per core (16 own + 2 halo)
NB = 7               # W pages
PW = 25              # stored page width (23 real + 2 halo)
STRIDE = 23          # page stride in real-w
P = ROWS * NB        # 126 partitions
WIN = 100            # time steps per window
CE = 10              # steps per E/sigmoid chunk (50*CE <= 512 PSUM bank)
CH = 20              # steps per conv chunk (23*CH <= 512)
PAD = np.float32(1.0e6)

_OP = None


def _register_op():
    global _OP
    if _OP is not None:
        return _OP
    from concourse import dve_ops
    from concourse.dve_spec import Spec, Src0, Src1, C0, C1, select, lower
    from concourse.dve_uop import DveOpSpec

    name = "SVS_UPDATE_ANT"
    for o in dve_ops.OPS:
        if o.name == name:
            _OP = o
            return o
    spec = Spec(
        body=select(Src0 > Src1, Src1 + C0, Src1 - C1),
        reference=lambda in0, in1, c0, c1, c2: np.where(
            in0 > in1,
            (in1 + np.float32(c0)).astype(np.float32),
            (in1 - np.float32(c1)).astype(np.float32),
        ).astype(np.float32),
    )
    opcode = dve_ops._CUSTOM_DVE_ROW_BASE + len(dve_ops.OPS)
    shas = {}
    for ver in ("v3", "v4"):
        uops = lower(spec, ver=ver)
        shas[ver] = DveOpSpec(name=name, opcode=opcode, uops=uops, rd1_en=True).sha(ver)
    op = dve_ops.DveOp(name, spec, subdim=False, uops_sha=shas)
    dve_ops.OPS.append(op)
    dve_ops._SUB_OPCODE_FOR_NAME[name] = opcode
    dve_ops.CUSTOM_DVE_SPECS[name] = spec
    _OP = op
    return op


def _build_program(d_open, d_close, hc, hbias, relu_bias):
    """One SPMD Bass program (same instruction stream on all 8 cores)."""
    from concourse import mybir, tile, bacc

    op = _register_op()
    nc = bacc.Bacc("TRN2", target_bir_lowering=False, debug=False,
                   num_devices=NCORES)
    f32 = mybir.dt.float32
    f16 = mybir.dt.float16
    xp_d = nc.dram_tensor("xp", [P, T, 2 * PW], f32, kind="ExternalInput").ap()
    s0_d = nc.dram_tensor("s0", [P, 2 * PW], f32, kind="ExternalInput").ap()
    eye_d = nc.dram_tensor("eye", [2, P, P], f32, kind="ExternalInput").ap()
    band_d = nc.dram_tensor("band", [3, P, P], f16, kind="ExternalInput").ap()
    out_d = nc.dram_tensor("out", [P, T, STRIDE], f32, kind="ExternalOutput").ap()

    Sig = mybir.ActivationFunctionType.Sigmoid
    Relu = mybir.ActivationFunctionType.Relu
    FD = 2 * PW
    NW = T // WIN

    with tile.TileContext(nc) as tc:
        with (
            tc.tile_pool(name="consts", bufs=1) as cpool,
            tc.tile_pool(name="x2", bufs=2) as x2pool,
            tc.tile_pool(name="traj", bufs=2) as tpool,
            tc.tile_pool(name="th", bufs=2) as thpool,
            tc.tile_pool(name="outw", bufs=2) as opool,
            tc.tile_pool(name="eps", bufs=3, space="PSUM") as epspool,
            tc.tile_pool(name="cps", bufs=3, space="PSUM") as cpspool,
        ):
            eyes = cpool.tile([P, 2 * P], f32)
            nc.sync.dma_start(eyes[:].rearrange("p (d q) -> p d q", d=2),
                              eye_d.rearrange("d p q -> p d q"))
            bands = cpool.tile([P, 3 * P], f16)
            nc.sync.dma_start(bands[:].rearrange("p (d q) -> p d q", d=3),
                              band_d.rearrange("d p q -> p d q"))
            hbias_t = cpool.tile([P, 1], f32)
            nc.vector.memset(hbias_t[:], hbias)
            rbias_t = cpool.tile([P, 1], f32)
            nc.vector.memset(rbias_t[:], relu_bias)

            trajs = []
            for w in range(NW):
                traj = tpool.tile([P, FD * (WIN + 1)], f32, tag="traj")
                trajs.append(traj)
                x2 = x2pool.tile([P, FD * WIN], f32, tag="x2")
                nc.sync.dma_start(
                    x2[:].rearrange("p (t f) -> p t f", t=WIN),
                    xp_d[:, w * WIN:(w + 1) * WIN, :],
                )
                # carry into slot 0
                if w == 0:
                    nc.sync.dma_start(traj[:, 0:FD], s0_d[:])
                else:
                    nc.vector.tensor_copy(
                        traj[:, 0:FD], trajs[w - 1][:, FD * WIN:FD * (WIN + 1)]
                    )
                # the chain: one fused VectorE op per time step
                for i in range(WIN):
                    nc.vector._custom_dve(
                        op,
                        out=traj[:, FD * (i + 1):FD * (i + 2)],
                        in0=x2[:, FD * i:FD * (i + 1)],
                        in1=traj[:, FD * i:FD * (i + 1)],
                        s0=d_open,
                        s1=d_close,
                    )
                # E = hc*x2 - hc*S_pre on TensorE; sigmoid from PSUM on ScalarE
                th = thpool.tile([P, FD * WIN], f16, tag="th")
                for c in range(0, WIN, CE):
                    eps = epspool.tile([P, FD * CE], f32, tag="eps")
                    nc.tensor.matmul(
                        eps[:], eyes[:, 0:P],
                        x2[:, FD * c:FD * (c + CE)], start=True, stop=False,
                    )
                    nc.tensor.matmul(
                        eps[:], eyes[:, P:2 * P],
                        traj[:, FD * c:FD * (c + CE)], start=False, stop=True,
                    )
                    nc.scalar.activation(
                        th[:, FD * c:FD * (c + CE)], eps[:], Sig,
                        bias=hbias_t[:], scale=1.0,
                    )
                # conv: 6 banded fp16 matmuls per chunk (th + tl folded in)
                outw = opool.tile([P, STRIDE * WIN], f32, tag="outw")
                for cs in range(0, WIN, CH):
                    ps = cpspool.tile([P, STRIDE * CH], f32, tag="cps")
                    tchunk = th[:, FD * cs:FD * (cs + CH)].rearrange(
                        "p (t f) -> p t f", t=CH
                    )
                    k = 0
                    for half in (0, PW):
                        for dx in (-1, 0, 1):
                            o = half + 1 + dx
                            nc.tensor.matmul(
                                ps[:].rearrange("p (t f) -> p t f", t=CH),
                                bands[:, (dx + 1) * P:(dx + 2) * P],
                                tchunk[:, :, o:o + STRIDE],
                                start=(k == 0), stop=(k == 5),
                            )
                            k += 1
                    nc.scalar.activation(
                        outw[:, STRIDE * cs:STRIDE * (cs + CH)],
                        ps[:], Relu, bias=rbias_t[:], scale=1.0,
                    )
                ws = w * WIN
                nc.sync.dma_start(
                    out_d[:, ws:ws + WIN, :],
                    outw[:].rearrange("p (t j) -> p t j", j=STRIDE),
                )
    nc.compile()
    return nc


_PROG_CACHE = {}


def _get_program(key, d_open, d_close, hc, hbias, relu_bias):
    if key not in _PROG_CACHE:
        _PROG_CACHE[key] = _build_program(d_open, d_close, hc, hbias, relu_bias)
    return _PROG_CACHE[key]


def _prep_inputs(x, params, ht0, lt0, kern, hc):
    """Build per-core input maps (host-side sharding)."""
    x = np.ascontiguousarray(x.reshape(T, H, W).astype(np.float32))
    ht0 = ht0.astype(np.float32)
    lt0 = lt0.astype(np.float32)
    kern = kern.astype(np.float32)

    # padded frame: rows [-1, H], cols [-1, W+2), pad value 1e6
    xp = np.full((T, H + 2, W + 3), PAD, np.float32)
    xp[:, 1:H + 1, 1:W + 1] = x
    hp = np.zeros((H + 2, W + 3), np.float32)
    hp[1:H + 1, 1:W + 1] = ht0
    lp = np.zeros((H + 2, W + 3), np.float32)
    lp[1:H + 1, 1:W + 1] = -lt0

    # E matrices: hc*I and -hc*I
    eye = np.zeros((2, P, P), np.float32)
    eye[0] = np.eye(P, dtype=np.float32) * np.float32(hc)
    eye[1] = np.eye(P, dtype=np.float32) * np.float32(-hc)
    # band matrices: band[dx][p_in, p_out] = k[h_in-h_out+1, dx+1]
    band = np.zeros((3, P, P), np.float16)
    for dxi in range(3):
        for h_out in range(ROWS):
            for dy in (-1, 0, 1):
                h_in = h_out + dy
                if 0 <= h_in < ROWS:
                    for wb in range(NB):
                        band[dxi, h_in * NB + wb, h_out * NB + wb] = kern[dy + 1, dxi]

    in_maps = []
    for c in range(NCORES):
        r0 = 16 * c
        xc = np.empty((ROWS, NB, T, 2 * PW), np.float32)
        sc = np.empty((ROWS, NB, 2 * PW), np.float32)
        for wb in range(NB):
            c0 = STRIDE * wb
            blk = xp[:, r0:r0 + ROWS, c0:c0 + PW].transpose(1, 0, 2)
            xc[:, wb, :, 0:PW] = blk
            xc[:, wb, :, PW:2 * PW] = -blk
            sc[:, wb, 0:PW] = hp[r0:r0 + ROWS, c0:c0 + PW]
            sc[:, wb, PW:2 * PW] = lp[r0:r0 + ROWS, c0:c0 + PW]
        in_maps.append({
            "xp": np.ascontiguousarray(xc.reshape(P, T, 2 * PW)),
            "s0": np.ascontiguousarray(sc.reshape(P, 2 * PW)),
            "eye": eye,
            "band": band,
        })
    return in_maps


TRACE = False        # test-harness hook: profile the SPMD run
LAST_RESULT = None


def kernel(x, params, ht0, lt0, kernel):
    global LAST_RESULT
    from concourse.bass_utils import run_bass_kernel_spmd

    p = np.asarray(params, np.float32)
    d_close, d_open, d_hot, hc = (float(p[0]), float(p[1]), float(p[2]), float(p[3]))
    kern = np.asarray(kernel, np.float32)
    hbias = float(np.float32(-np.float32(d_hot) * np.float32(hc)))
    relu_bias = float(np.float32(1.0) - np.float32(kern.sum()))

    key = (d_close, d_open, d_hot, hc, kern.tobytes())
    nc = _get_program(key, d_open, d_close, hc, hbias, relu_bias)
    in_maps = _prep_inputs(np.asarray(x), p, np.asarray(ht0), np.asarray(lt0),
                           kern, hc)
    r = run_bass_kernel_spmd(nc, in_maps, list(range(NCORES)), trace=TRACE)
    LAST_RESULT = r
    res = r.results
    out = np.empty((T, H, W), np.float32)
    for c in range(NCORES):
        out[:, 16 * c:16 * (c + 1), :] = _assemble(res[c]["out"])
    return out.reshape(T, 1, H, W).astype(np.float32)


def _assemble(raw):
    """[P, T, STRIDE] staging -> [T, 16, W] (drop halo rows h=0,17, pad cols)."""
    v = raw.reshape(ROWS, NB, T, STRIDE)[1:17]  # own rows
    full = v.transpose(2, 0, 1, 3).reshape(T, 16, NB * STRIDE)
    return full[:, :, :W]


# revision 21
# speedup vs baseline: 3.2012x; 1.0146x over previous
"""Trainium2 Bass kernel for nn_AugmentableSVSAlgorithm (scatter_memory).

Reference semantics:
  per-frame recurrence over T=2000 frames with carry (ht, lt) [128,160]:
      th = sigmoid((x - ht - d_hot) * hc);  tl = sigmoid((lt - x - d_hot) * hc)
      ht' = ht + d_open if x > ht else ht - d_close
      lt' = lt - d_open if lt > x else lt + d_close
      hot = th + tl
  then out = relu(1 - conv3x3(1 - hot, k, pad=1)).

Implementation (8 NeuronCores, SPMD, H-split sharding):
  * Core c owns rows [16c, 16c+16), processes rows [16c-1, 16c+17) (halo).
    Out-of-range halo rows and W-pad columns carry x = +1e6, which saturates
    both sigmoids so hot == 1.0 exactly — reproducing the reference conv's
    zero-padding of (1 - hot) with no edge cases anywhere.
  * State pairs S = [h | L], L = -lt: both thresholds share the update
    S' = select(x2 > S, S + d_open, S - d_close), x2 = [x | -x] (interleaved
    on the host). One fused custom VectorE op per time step — branch
    decisions and state arithmetic bit-exact vs the reference.
  * Layout per core: 126 partitions = (h:18, wb:7), pages of 25 cols
    (23-col stride + 2 halo cols) so the conv never crosses pages in the
    free dim. H direction on partitions via TensorE banded matrices.
  * VectorE runs ONLY the chain. E = hc*x2 - hc*S is produced by TensorE
    (scaled identity matmuls) into PSUM; ScalarE applies sigmoid straight
    from PSUM (fp16 out). The th+tl add is folded into the conv by
    linearity (6 banded fp16 matmuls per chunk). GpSimd does nothing —
    it would contend with VectorE for the shared SBUF port.
"""

import numpy as np

T, H, W = 2000, 128, 160
NCORES = 8
ROWS = 18            # rows per core (16 own + 2 halo)
NB = 7               # W pages
PW = 25              # stored page width (23 real + 2 halo)
STRIDE = 23          # page stride in real-w
P = ROWS * NB        # 126 partitions
WIN = 100            # time steps per window
CE = 10              # steps per E/sigmoid chunk (50*CE <= 512 PSUM bank)
CH = 20              # steps per conv chunk (23*CH <= 512)
PAD = np.float32(1.0e6)

_OP = None


def _register_op():
    global _OP
    if _OP is not None:
        return _OP
    from concourse import dve_ops
    from concourse.dve_spec import Spec, Src0, Src1, C0, C1, select, lower
    from concourse.dve_uop import DveOpSpec

    name = "SVS_UPDATE_ANT"
    for o in dve_ops.OPS:
        if o.name == name:
            _OP = o
            return o
    spec = Spec(
        body=select(Src0 > Src1, Src1 + C0, Src1 - C1),
        reference=lambda in0, in1, c0, c1, c2: np.where(
            in0 > in1,
            (in1 + np.float32(c0)).astype(np.float32),
            (in1 - np.float32(c1)).astype(np.float32),
        ).astype(np.float32),
    )
    opcode = dve_ops._CUSTOM_DVE_ROW_BASE + len(dve_ops.OPS)
    shas = {}
    for ver in ("v3", "v4"):
        uops = lower(spec, ver=ver)
        shas[ver] = DveOpSpec(name=name, opcode=opcode, uops=uops, rd1_en=True).sha(ver)
    op = dve_ops.DveOp(name, spec, subdim=False, uops_sha=shas)
    dve_ops.OPS.append(op)
    dve_ops._SUB_OPCODE_FOR_NAME[name] = opcode
    dve_ops.CUSTOM_DVE_SPECS[name] = spec
    _OP = op
    return op


def _build_program(d_open, d_close, hc, hbias, relu_bias):
    """One SPMD Bass program (same instruction stream on all 8 cores)."""
    from concourse import mybir, tile, bacc

    op = _register_op()
    nc = bacc.Bacc("TRN2", target_bir_lowering=False, debug=False,
                   num_devices=NCORES)
    f32 = mybir.dt.float32
    f16 = mybir.dt.float16
    xp_d = nc.dram_tensor("xp", [P, T, 2 * PW], f32, kind="ExternalInput").ap()
    s0_d = nc.dram_tensor("s0", [P, 2 * PW], f32, kind="ExternalInput").ap()
    eye_d = nc.dram_tensor("eye", [2, P, P], f32, kind="ExternalInput").ap()
    band_d = nc.dram_tensor("band", [3, P, P], f16, kind="ExternalInput").ap()
    out_d = nc.dram_tensor("out", [P, T, STRIDE], f32, kind="ExternalOutput").ap()

    Sig = mybir.ActivationFunctionType.Sigmoid
    Relu = mybir.ActivationFunctionType.Relu
    FD = 2 * PW
    # smaller first windows (shorter initial DMA before the chain can start)
    # and last windows (shorter post-chain tail)
    if T >= 4 * WIN and WIN % 2 == 0:
        wins = [WIN // 2, WIN // 2] + [WIN] * (T // WIN - 2) + [WIN // 2, WIN // 2]
    else:
        wins = [WIN] * (T // WIN)
    assert sum(wins) == T

    with tile.TileContext(nc) as tc:
        with (
            tc.tile_pool(name="consts", bufs=1) as cpool,
            tc.tile_pool(name="x2", bufs=2) as x2pool,
            tc.tile_pool(name="traj", bufs=2) as tpool,
            tc.tile_pool(name="th", bufs=2) as thpool,
            tc.tile_pool(name="outw", bufs=2) as opool,
            tc.tile_pool(name="eps", bufs=3, space="PSUM") as epspool,
            tc.tile_pool(name="cps", bufs=3, space="PSUM") as cpspool,
        ):
            eyes = cpool.tile([P, 2 * P], f32)
            nc.sync.dma_start(eyes[:].rearrange("p (d q) -> p d q", d=2),
                              eye_d.rearrange("d p q -> p d q"))
            bands = cpool.tile([P, 3 * P], f16)
            nc.sync.dma_start(bands[:].rearrange("p (d q) -> p d q", d=3),
                              band_d.rearrange("d p q -> p d q"))
            hbias_t = cpool.tile([P, 1], f32)
            nc.vector.memset(hbias_t[:], hbias)
            rbias_t = cpool.tile([P, 1], f32)
            nc.vector.memset(rbias_t[:], relu_bias)

            trajs = []
            t_base = 0
            for w, wlen in enumerate(wins):
                traj = tpool.tile([P, FD * (WIN + 1)], f32, tag="traj")
                trajs.append(traj)
                x2 = x2pool.tile([P, FD * WIN], f32, tag="x2")
                nc.sync.dma_start(
                    x2[:, 0:FD * wlen].rearrange("p (t f) -> p t f", t=wlen),
                    xp_d[:, t_base:t_base + wlen, :],
                )
                # carry into slot 0
                if w == 0:
                    nc.sync.dma_start(traj[:, 0:FD], s0_d[:])
                else:
                    pl = wins[w - 1]
                    nc.vector.tensor_copy(
                        traj[:, 0:FD], trajs[w - 1][:, FD * pl:FD * (pl + 1)]
                    )
                # the chain: one fused VectorE op per time step
                for i in range(wlen):
                    nc.vector._custom_dve(
                        op,
                        out=traj[:, FD * (i + 1):FD * (i + 2)],
                        in0=x2[:, FD * i:FD * (i + 1)],
                        in1=traj[:, FD * i:FD * (i + 1)],
                        s0=d_open,
                        s1=d_close,
                    )
                # E = hc*x2 - hc*S_pre on TensorE; sigmoid from PSUM on ScalarE
                th = thpool.tile([P, FD * WIN], f16, tag="th")
                for c in range(0, wlen, CE):
                    ce = min(CE, wlen - c)
                    eps = epspool.tile([P, FD * CE], f32, tag="eps")
                    nc.tensor.matmul(
                        eps[:, 0:FD * ce], eyes[:, 0:P],
                        x2[:, FD * c:FD * (c + ce)], start=True, stop=False,
                    )
                    nc.tensor.matmul(
                        eps[:, 0:FD * ce], eyes[:, P:2 * P],
                        traj[:, FD * c:FD * (c + ce)], start=False, stop=True,
                    )
                    nc.scalar.activation(
                        th[:, FD * c:FD * (c + ce)], eps[:, 0:FD * ce], Sig,
                        bias=hbias_t[:], scale=1.0,
                    )
                # conv: 6 banded fp16 matmuls per chunk (th + tl folded in)
                outw = opool.tile([P, STRIDE * WIN], f32, tag="outw")
                for cs in range(0, wlen, CH):
                    ch = min(CH, wlen - cs)
                    ps = cpspool.tile([P, STRIDE * CH], f32, tag="cps")
                    tchunk = th[:, FD * cs:FD * (cs + ch)].rearrange(
                        "p (t f) -> p t f", t=ch
                    )
                    k = 0
                    for half in (0, PW):
                        for dx in (-1, 0, 1):
                            o = half + 1 + dx
                            nc.tensor.matmul(
                                ps[:, 0:STRIDE * ch].rearrange(
                                    "p (t f) -> p t f", t=ch
                                ),
                                bands[:, (dx + 1) * P:(dx + 2) * P],
                                tchunk[:, :, o:o + STRIDE],
                                start=(k == 0), stop=(k == 5),
                            )
                            k += 1
                    nc.scalar.activation(
                        outw[:, STRIDE * cs:STRIDE * (cs + ch)],
                        ps[:, 0:STRIDE * ch], Relu, bias=rbias_t[:], scale=1.0,
                    )
                nc.sync.dma_start(
                    out_d[:, t_base:t_base + wlen, :],
                    outw[:, 0:STRIDE * wlen].rearrange("p (t j) -> p t j", j=STRIDE),
                )
                t_base += wlen
    nc.compile()
    return nc


_PROG_CACHE = {}


def _get_program(key, d_open, d_close, hc, hbias, relu_bias):
    if key not in _PROG_CACHE:
        _PROG_CACHE[key] = _build_program(d_open, d_close, hc, hbias, relu_bias)
    return _PROG_CACHE[key]


def _prep_inputs(x, params, ht0, lt0, kern, hc):
    """Build per-core input maps (host-side sharding)."""
    x = np.ascontiguousarray(x.reshape(T, H, W).astype(np.float32))
    ht0 = ht0.astype(np.float32)
    lt0 = lt0.astype(np.float32)
    kern = kern.astype(np.float32)

    # padded frame: rows [-1, H], cols [-1, W+2), pad value 1e6
    xp = np.full((T, H + 2, W + 3), PAD, np.float32)
    xp[:, 1:H + 1, 1:W + 1] = x
    hp = np.zeros((H + 2, W + 3), np.float32)
    hp[1:H + 1, 1:W + 1] = ht0
    lp = np.zeros((H + 2, W + 3), np.float32)
    lp[1:H + 1, 1:W + 1] = -lt0

    # E matrices: hc*I and -hc*I
    eye = np.zeros((2, P, P), np.float32)
    eye[0] = np.eye(P, dtype=np.float32) * np.float32(hc)
    eye[1] = np.eye(P, dtype=np.float32) * np.float32(-hc)
    # band matrices: band[dx][p_in, p_out] = k[h_in-h_out+1, dx+1]
    band = np.zeros((3, P, P), np.float16)
    for dxi in range(3):
        for h_out in range(ROWS):
            for dy in (-1, 0, 1):
                h_in = h_out + dy
                if 0 <= h_in < ROWS:
                    for wb in range(NB):
                        band[dxi, h_in * NB + wb, h_out * NB + wb] = kern[dy + 1, dxi]

    in_maps = []
    for c in range(NCORES):
        r0 = 16 * c
        xc = np.empty((ROWS, NB, T, 2 * PW), np.float32)
        sc = np.empty((ROWS, NB, 2 * PW), np.float32)
        for wb in range(NB):
            c0 = STRIDE * wb
            blk = xp[:, r0:r0 + ROWS, c0:c0 + PW].transpose(1, 0, 2)
            xc[:, wb, :, 0:PW] = blk
            xc[:, wb, :, PW:2 * PW] = -blk
            sc[:, wb, 0:PW] = hp[r0:r0 + ROWS, c0:c0 + PW]
            sc[:, wb, PW:2 * PW] = lp[r0:r0 + ROWS, c0:c0 + PW]
        in_maps.append({
            "xp": np.ascontiguousarray(xc.reshape(P, T, 2 * PW)),
            "s0": np.ascontiguousarray(sc.reshape(P, 2 * PW)),
            "eye": eye,
            "band": band,
        })
    return in_maps


TRACE = False        # test-harness hook: profile the SPMD run
LAST_RESULT = None


def kernel(x, params, ht0, lt0, kernel):
    global LAST_RESULT
    from concourse.bass_utils import run_bass_kernel_spmd

    p = np.asarray(params, np.float32)
    d_close, d_open, d_hot, hc = (float(p[0]), float(p[1]), float(p[2]), float(p[3]))
    kern = np.asarray(kernel, np.float32)
    hbias = float(np.float32(-np.float32(d_hot) * np.float32(hc)))
    relu_bias = float(np.float32(1.0) - np.float32(kern.sum()))

    key = (d_close, d_open, d_hot, hc, kern.tobytes())
    nc = _get_program(key, d_open, d_close, hc, hbias, relu_bias)
    in_maps = _prep_inputs(np.asarray(x), p, np.asarray(ht0), np.asarray(lt0),
                           kern, hc)
    r = run_bass_kernel_spmd(nc, in_maps, list(range(NCORES)), trace=TRACE)
    LAST_RESULT = r
    res = r.results
    out = np.empty((T, H, W), np.float32)
    for c in range(NCORES):
        out[:, 16 * c:16 * (c + 1), :] = _assemble(res[c]["out"])
    return out.reshape(T, 1, H, W).astype(np.float32)


def _assemble(raw):
    """[P, T, STRIDE] staging -> [T, 16, W] (drop halo rows h=0,17, pad cols)."""
    v = raw.reshape(ROWS, NB, T, STRIDE)[1:17]  # own rows
    full = v.transpose(2, 0, 1, 3).reshape(T, 16, NB * STRIDE)
    return full[:, :, :W]
